# revision 20
# baseline (speedup 1.0000x reference)
"""BoundaryGCN on 8 Trainium2 NeuronCores.

Strategy (dst-sharded, memory-roofline oriented):
  - Nodes are row-sharded by id across the 8 cores (12500/core, padded to
    12544 = 98*128). All per-node compute (input linear, W1/W2 matmuls,
    LayerNorm, residual, output linear) is data-parallel over node blocks.
  - Per layer, each core computes its block of m = h @ W1 + b1, pre-scales
    rows by inv = rsqrt(deg+1) and AllGathers the scaled table
    (symmetric GCN norm factorizes: coef = inv[src]*inv[dst], so the
    per-edge coefficient disappears entirely).
  - Edges are sharded by destination core. Each core's edges are sorted by
    (src-group, dst-window) host-side; messages are fetched with the SWDGE
    dma_gather custom instruction (512B rows) and segment-summed on the PE
    via one-hot matmuls accumulated in PSUM, then added into an
    SBUF-resident aggregate. Self-loops are folded into the aggregate's
    initialization (exact, no gather traffic).
  - agg is post-scaled by inv, pushed through W2, residual+LN+ReLU, and the
    final LN + output projection produce each core's [12500, 64] slice.

The per-edge index/one-hot metadata is precomputed on the host from
edge_index (it is the same for all 3 layers) and shipped as kernel inputs;
the instruction stream itself is identical across cores (SPMD).
"""
import sys

sys.path.insert(0, "/opt/trn_rl_repo")

import numpy as np

import concourse.bacc as bacc
import concourse.mybir as mybir
import concourse.tile as tile
from concourse.bass_utils import run_bass_kernel_spmd

F32 = mybir.dt.float32
I16 = mybir.dt.int16


# ---------------------------------------------------------------------------
# walrus in this toolchain rejects >1 sync wait per instruction; split extras
# onto same-engine NoOps placed right before the instruction.
def _split_multi_waits(nc, maxw=1):
    n = 0
    for fn in nc.m.functions:
        for bb in fn.blocks:
            insts = bb.instructions
            i = 0
            while i < len(insts):
                ins = insts[i]
                si = ins.sync_info
                if si is None or len(si.on_wait) <= maxw:
                    i += 1
                    continue
                waits = list(si.on_wait)
                keep = waits[-maxw:]
                rest = waits[:-maxw]
                del si.on_wait[:]
                si.on_wait.extend(keep)
                for j in range(0, len(rest), maxw):
                    n += 1
                    nop = mybir.InstNoOp(
                        name=f"wsplit-{n}",
                        sync_info=mybir.SyncInfo(
                            on_wait=list(rest[j:j + maxw]), on_update=[]
                        ),
                        bass_nofuse=True,
                        engine=ins.engine,
                        ins=[],
                        outs=[],
                    )
                    try:
                        nc.register_instruction(nop, overwrite=True)
                    except Exception:
                        pass
                    insts.insert(i, nop)
                    i += 1
                i += 1
    return n


# ---------------------------------------------------------------------------
class Cfg:
    def __init__(self, N=100000, E=1600000, IN_DIM=128, EMB=128, HID=128,
                 OUT=64, L=3, NCORES=8, CH=1024, table_bf16=False):
        assert EMB == 128 and HID == 128 and IN_DIM == 128
        self.N, self.E = N, E
        self.IN_DIM, self.EMB, self.HID, self.OUT, self.L = IN_DIM, EMB, HID, OUT, L
        self.NCORES = NCORES
        self.B_REAL = N // NCORES                      # real nodes per core
        self.NW = (self.B_REAL + 127) // 128           # dst windows per core
        self.B = self.NW * 128                         # padded nodes per core
        self.NPAD = NCORES * self.B
        self.RANKS_PER_GROUP = 2
        self.GROUPS = NCORES // self.RANKS_PER_GROUP
        self.GROUP_ROWS = self.RANKS_PER_GROUP * self.B
        assert self.GROUP_ROWS <= 32767
        self.CH = CH                                   # gather chunk (tokens)
        self.LN_EPS = 1e-5
        self.table_dt = mybir.dt.bfloat16 if table_bf16 else F32
        self.table_np = np.dtype("bfloat16") if table_bf16 else np.float32


class EdgePlan:
    """Host-side uniform (SPMD) token-stream layout for the edge gather."""

    def __init__(self, cfg: Cfg, edge_index: np.ndarray):
        c = cfg
        src = np.asarray(edge_index[0], dtype=np.int64)
        dst = np.asarray(edge_index[1], dtype=np.int64)
        core = dst // c.B_REAL
        ldst = dst - core * c.B_REAL
        w = ldst >> 7                                  # dst window
        ohv = (ldst & 127).astype(np.float32)
        r = src // c.B_REAL
        l = src - r * c.B_REAL
        trow = r * c.B + l
        g = r // c.RANKS_PER_GROUP
        grow = (trow - g * c.GROUP_ROWS).astype(np.int64)

        cnt = np.zeros((c.NCORES, c.GROUPS, c.NW), np.int64)
        np.add.at(cnt, (core, g, w), 1)
        self.Tbar = np.ceil(cnt.max(axis=0) / 128).astype(np.int64)  # [G, NW]

        # global tile index layout: g-major, then w
        tiles_per_gw = self.Tbar                        # [G, NW]
        self.NTILES = int(tiles_per_gw.sum())
        self.TOK = self.NTILES * 128
        tile_start = np.zeros((c.GROUPS, c.NW), np.int64)
        acc = 0
        self.tiles_meta = []  # (g, w, start, stop) per global tile
        for gg in range(c.GROUPS):
            for ww in range(c.NW):
                tile_start[gg, ww] = acc
                T = int(tiles_per_gw[gg, ww])
                for t in range(T):
                    self.tiles_meta.append((gg, ww, t == 0, t == T - 1))
                acc += T
        self.group_tok = tiles_per_gw.sum(axis=1) * 128  # tokens per group
        self.tok_start = tile_start * 128               # [G, NW] token offset

        # per-core token arrays
        order = np.lexsort((src, w, g, core))
        core_s, g_s, w_s, grow_s, ohv_s = (
            core[order], g[order], w[order], grow[order], ohv[order])
        self.gidx = np.zeros((c.NCORES, self.TOK), np.int16)
        self.ohv = np.full((c.NCORES, self.TOK), -1.0, np.float32)
        # segment offsets per (core, g, w)
        seg_cnt = cnt  # [NCORES, G, NW]
        for cc in range(c.NCORES):
            sel = core_s == cc
            gg_, ww_, gr_, ov_ = g_s[sel], w_s[sel], grow_s[sel], ohv_s[sel]
            # within the sorted order, edges of (g,w) are contiguous
            seg_sizes = seg_cnt[cc].ravel()             # g-major, w within
            starts = self.tok_start.ravel()
            pos = np.repeat(starts, seg_sizes) + _ranges(seg_sizes)
            self.gidx[cc, pos] = gr_.astype(np.int16)
            self.ohv[cc, pos] = ov_

        # chunk split per group
        self.chunks = []  # (idx_free_off, tok_off, n_tok, group)
        idx_off = 0
        tok_off = 0
        for gg in range(c.GROUPS):
            rem = int(self.group_tok[gg])
            while rem > 0:
                n = min(c.CH, rem)
                self.chunks.append((idx_off, tok_off, n, gg))
                idx_off += n // 16
                tok_off += n
                rem -= n

        # wrapped idx layout [128, TOK//16]: per chunk block [16, n/16]
        # replicated 8x over partitions
        self.gidx_w = np.zeros((c.NCORES, 128, self.TOK // 16), np.int16)
        for (ioff, toff, n, gg) in self.chunks:
            blk = self.gidx[:, toff:toff + n].reshape(c.NCORES, n // 16, 16)
            blk = np.transpose(blk, (0, 2, 1))          # [NCORES, 16, n/16]
            self.gidx_w[:, :, ioff:ioff + n // 16] = np.tile(blk, (1, 8, 1))
        # ohv arranged [128, NTILES]
        self.ohv_sb = np.transpose(
            self.ohv.reshape(c.NCORES, self.NTILES, 128), (0, 2, 1)).copy()


def _ranges(sizes):
    """concat(arange(s) for s in sizes)"""
    total = int(sizes.sum())
    out = np.arange(total, dtype=np.int64)
    ends = np.cumsum(sizes)
    starts = ends - sizes
    out -= np.repeat(starts, sizes)
    return out


# ---------------------------------------------------------------------------
def build_program(cfg: Cfg, plan: EdgePlan):
    c = cfg
    TD = c.table_dt
    nc = bacc.Bacc("TRN2", target_bir_lowering=False, debug=False,
                   num_devices=c.NCORES)

    # ---- kernel I/O ----
    xT_d = nc.declare_dram_parameter("xT", [128, c.B], F32, isOutput=False)
    inv_d = nc.declare_dram_parameter("inv", [128, c.NW], F32, isOutput=False)
    gidx_d = nc.declare_dram_parameter("gidx", [128, plan.TOK // 16], I16, isOutput=False)
    ohv_d = nc.declare_dram_parameter("ohv", [128, plan.NTILES], TD, isOutput=False)
    iota_d = nc.declare_dram_parameter("iota", [128, 128], TD, isOutput=False)
    ident_d = nc.declare_dram_parameter("ident", [128, 128], F32, isOutput=False)
    Win_d = nc.declare_dram_parameter("Win", [128, c.EMB], F32, isOutput=False)
    b_in_d = nc.declare_dram_parameter("b_in", [128, 1], F32, isOutput=False)
    W1_d = nc.declare_dram_parameter("W1", [c.L * 128, c.HID], F32, isOutput=False)
    b1_d = nc.declare_dram_parameter("b1", [128, c.L], F32, isOutput=False)
    W2_d = nc.declare_dram_parameter("W2", [c.L * 128, c.EMB], F32, isOutput=False)
    b2_d = nc.declare_dram_parameter("b2", [128, c.L], F32, isOutput=False)
    lng_d = nc.declare_dram_parameter("lng", [128, c.L * 128], F32, isOutput=False)
    lnb_d = nc.declare_dram_parameter("lnb", [128, c.L * 128], F32, isOutput=False)
    outg_d = nc.declare_dram_parameter("outg", [128, 128], F32, isOutput=False)
    outb_d = nc.declare_dram_parameter("outb", [128, 128], F32, isOutput=False)
    Wout_d = nc.declare_dram_parameter("Wout", [128, c.OUT], F32, isOutput=False)
    bout_d = nc.declare_dram_parameter("bout", [c.OUT, 1], F32, isOutput=False)
    y_d = nc.declare_dram_parameter("y", [c.B, c.OUT], F32, isOutput=True)

    # chunking of the node dim for 512-wide matmuls
    mm_chunks = []
    off = 0
    while off < c.B:
        n = min(512, c.B - off)
        mm_chunks.append((off, n))
        off += n

    with tile.TileContext(nc) as tc:
        with (
            tc.tile_pool(name="const", bufs=1) as constp,
            tc.tile_pool(name="big", bufs=1) as bigp,
            tc.tile_pool(name="chunkp", bufs=3) as chunkp,
            tc.tile_pool(name="datap", bufs=2) as datap,
            tc.tile_pool(name="idxp", bufs=2) as idxp,
            tc.tile_pool(name="ohp", bufs=2) as ohp,
            tc.tile_pool(name="smallp", bufs=3) as smallp,
            tc.tile_pool(name="colp", bufs=3) as colp,
            tc.tile_pool(name="pmm", bufs=2, space="PSUM") as pmm,
            tc.tile_pool(name="pacc", bufs=2, space="PSUM") as pacc,
            tc.tile_pool(name="ptr", bufs=3, space="PSUM") as ptr,
            tc.tile_pool(name="dram", bufs=1, space="DRAM") as dramp,
        ):
            # ---- persistent SBUF ----
            hT = bigp.tile([128, c.B], F32, tag="hT")
            agg = bigp.tile([128, c.B], F32, tag="agg")
            ohv_sb = bigp.tile([128, plan.NTILES], TD, tag="ohv")
            inv_sb = constp.tile([128, c.NW], F32)
            iota_sb = constp.tile([128, 128], TD)
            ident_sb = constp.tile([128, 128], F32)
            Win_sb = constp.tile([128, c.EMB], F32)
            b_in_sb = constp.tile([128, 1], F32)
            W1_sb = constp.tile([128, c.L * c.HID], F32)
            b1_sb = constp.tile([128, c.L], F32)
            W2_sb = constp.tile([128, c.L * c.EMB], F32)
            b2_sb = constp.tile([128, c.L], F32)
            lng_sb = constp.tile([128, c.L * 128], F32)
            lnb_sb = constp.tile([128, c.L * 128], F32)
            outg_sb = constp.tile([128, 128], F32)
            outb_sb = constp.tile([128, 128], F32)
            Wout_sb = constp.tile([128, c.OUT], F32)
            bout_sb = constp.tile([c.OUT, 1], F32)
            eps_sb = constp.tile([128, 1], F32)
            nc.vector.memset(eps_sb[:], c.LN_EPS)

            nc.sync.dma_start(inv_sb[:], inv_d[:])
            nc.sync.dma_start(iota_sb[:], iota_d[:])
            nc.sync.dma_start(ident_sb[:], ident_d[:])
            nc.sync.dma_start(Win_sb[:], Win_d[:])
            nc.sync.dma_start(b_in_sb[:], b_in_d[:])
            for i in range(c.L):
                nc.sync.dma_start(W1_sb[:, i * c.HID:(i + 1) * c.HID],
                                  W1_d[i * 128:(i + 1) * 128, :])
                nc.sync.dma_start(W2_sb[:, i * c.EMB:(i + 1) * c.EMB],
                                  W2_d[i * 128:(i + 1) * 128, :])
            nc.sync.dma_start(b1_sb[:], b1_d[:])
            nc.sync.dma_start(b2_sb[:], b2_d[:])
            nc.sync.dma_start(lng_sb[:], lng_d[:])
            nc.sync.dma_start(lnb_sb[:], lnb_d[:])
            nc.sync.dma_start(outg_sb[:], outg_d[:])
            nc.sync.dma_start(outb_sb[:], outb_d[:])
            nc.sync.dma_start(Wout_sb[:], Wout_d[:])
            nc.sync.dma_start(bout_sb[:], bout_d[:])
            nc.sync.dma_start(ohv_sb[:], ohv_d[:])

            # ---- DRAM tiles for the collective (Shared tiles are
            # single-write: one per layer) ----
            cc_ins = [dramp.tile([c.B, 128], TD, tag=f"cc_in{i}",
                                 name=f"cc_in{i}") for i in range(c.L)]
            m_tables = [dramp.tile([c.NPAD, 128], TD, addr_space="Shared",
                                   tag=f"m_table{i}", name=f"m_table{i}")
                        for i in range(c.L)]

            # ---- layer 0: hT = relu(x @ Win + b_in), feature-major ----
            for (off, n) in mm_chunks:
                xt = chunkp.tile([128, 512], F32, tag="xt")
                nc.sync.dma_start(xt[:, :n], xT_d[:, off:off + n])
                ps = pmm.tile([128, 512], F32, tag="psmm")
                nc.tensor.matmul(ps[:, :n], Win_sb[:], xt[:, :n],
                                 start=True, stop=True)
                nc.scalar.activation(hT[:, off:off + n], ps[:, :n],
                                     mybir.ActivationFunctionType.Relu,
                                     bias=b_in_sb[:, :1], scale=1.0)

            def layer_norm_tile(z_ps, k, g_ap, b_ap, relu, dest_ap):
                """z_ps: [128,128] PSUM node-major. Writes dest_ap (feature
                major [128,128] slice of an SBUF tensor) = T(LN(z)) (+relu)."""
                sum_ = colp.tile([128, 1], F32, tag="lnsum")
                nc.vector.tensor_reduce(sum_[:], z_ps[:], mybir.AxisListType.X,
                                        mybir.AluOpType.add)
                mean = colp.tile([128, 1], F32, tag="lnmean")
                nc.vector.tensor_scalar_mul(mean[:], sum_[:], 1.0 / 128.0)
                cen = smallp.tile([128, 128], F32, tag="lncen")
                nc.vector.tensor_scalar(cen[:], z_ps[:], mean[:, :1], None,
                                        op0=mybir.AluOpType.subtract)
                sq = smallp.tile([128, 128], F32, tag="lnsq")
                ss = colp.tile([128, 1], F32, tag="lnss")
                nc.scalar.activation(sq[:], cen[:],
                                     mybir.ActivationFunctionType.Square,
                                     accum_out=ss[:])
                sd = colp.tile([128, 1], F32, tag="lnsd")
                nc.scalar.activation(sd[:], ss[:],
                                     mybir.ActivationFunctionType.Sqrt,
                                     bias=eps_sb[:, :1], scale=1.0 / 128.0)
                rstd = colp.tile([128, 1], F32, tag="lnrstd")
                nc.vector.reciprocal(rstd[:], sd[:])
                nrm = smallp.tile([128, 128], F32, tag="lnnrm")
                nc.vector.tensor_scalar(nrm[:], cen[:], rstd[:, :1], None,
                                        op0=mybir.AluOpType.mult)
                ng = smallp.tile([128, 128], F32, tag="lnng")
                nc.vector.tensor_tensor(ng[:], nrm[:], g_ap,
                                        op=mybir.AluOpType.mult)
                hnew = smallp.tile([128, 128], F32, tag="lnhnew")
                nc.vector.tensor_tensor(hnew[:], ng[:], b_ap,
                                        op=mybir.AluOpType.add)
                pt = ptr.tile([128, 128], F32, tag="pstr")
                nc.tensor.transpose(pt[:], hnew[:], ident_sb[:])
                nc.scalar.activation(
                    dest_ap, pt[:],
                    mybir.ActivationFunctionType.Relu if relu
                    else mybir.ActivationFunctionType.Copy)

            # ---- GCN layers ----
            for li in range(c.L):
                W1l = W1_sb[:, li * c.HID:(li + 1) * c.HID]
                W2l = W2_sb[:, li * c.EMB:(li + 1) * c.EMB]
                cc_in = cc_ins[li]
                m_table = m_tables[li]

                # m^T chunks, bias, transpose to node-major, scale by inv,
                # write agg init and the AllGather input
                for (off, n) in mm_chunks:
                    ps = pmm.tile([128, 512], F32, tag="psmm")
                    nc.tensor.matmul(ps[:, :n], W1l, hT[:, off:off + n],
                                     start=True, stop=True)
                    mT = chunkp.tile([128, 512], F32, tag="mT")
                    nc.vector.tensor_scalar(mT[:, :n], ps[:, :n],
                                            b1_sb[:, li:li + 1], None,
                                            op0=mybir.AluOpType.add)
                    for kk in range(0, n, 128):
                        k = (off + kk) // 128
                        pt = ptr.tile([128, 128], F32, tag="pstr")
                        nc.tensor.transpose(pt[:], mT[:, kk:kk + 128],
                                            ident_sb[:])
                        # agg init = inv * m   (self-loop term, f32)
                        nc.vector.tensor_scalar(
                            agg[:, k * 128:(k + 1) * 128], pt[:],
                            inv_sb[:, k:k + 1], None,
                            op0=mybir.AluOpType.mult)
                        if TD == F32:
                            nc.sync.dma_start(
                                cc_in[k * 128:(k + 1) * 128, :],
                                agg[:, k * 128:(k + 1) * 128])
                        else:
                            mb = smallp.tile([128, 128], TD, tag="mbf")
                            nc.vector.tensor_scalar(
                                mb[:], pt[:], inv_sb[:, k:k + 1], None,
                                op0=mybir.AluOpType.mult)
                            nc.sync.dma_start(
                                cc_in[k * 128:(k + 1) * 128, :], mb[:])

                nc.gpsimd.collective_compute(
                    "AllGather", mybir.AluOpType.bypass,
                    ins=[cc_in.opt()], outs=[m_table.opt()],
                    replica_groups=[list(range(c.NCORES))])

                # gather + one-hot segment-sum
                tile_i = 0
                cur_ps = None
                for (ioff, toff, n_tok, gg) in plan.chunks:
                    it = idxp.tile([128, c.CH // 16], I16, tag="idx")
                    nc.sync.dma_start(it[:, :n_tok // 16],
                                      gidx_d[:, ioff:ioff + n_tok // 16])
                    dt_ = datap.tile([128, c.CH // 128, 128], TD, tag="gdat")
                    nc.gpsimd.dma_gather(
                        dt_[:, :n_tok // 128, :],
                        m_table[gg * c.GROUP_ROWS:(gg + 1) * c.GROUP_ROWS, :],
                        it[:, :n_tok // 16],
                        num_idxs=n_tok, num_idxs_reg=n_tok, elem_size=128)
                    n_t = n_tok // 128
                    OHW = 8
                    for tb in range(0, n_t, OHW):
                        te = min(tb + OHW, n_t)
                        oh = ohp.tile([128, OHW, 128], TD, tag="oh")
                        nc.vector.tensor_tensor(
                            oh[:, :te - tb, :],
                            iota_sb[:].unsqueeze(1).to_broadcast(
                                [128, te - tb, 128]),
                            ohv_sb[:, tile_i:tile_i + (te - tb)].unsqueeze(2)
                            .to_broadcast([128, te - tb, 128]),
                            op=mybir.AluOpType.is_equal)
                        for t in range(tb, te):
                            (g_, w_, is_start, is_stop) = \
                                plan.tiles_meta[tile_i + t - tb]
                            if is_start:
                                cur_ps = pacc.tile([128, 128], F32,
                                                   tag="psacc")
                            nc.tensor.matmul(cur_ps[:], oh[:, t - tb, :],
                                             dt_[:, t, :],
                                             start=is_start, stop=is_stop)
                            if is_stop:
                                nc.vector.tensor_tensor(
                                    agg[:, w_ * 128:(w_ + 1) * 128],
                                    agg[:, w_ * 128:(w_ + 1) * 128],
                                    cur_ps[:], op=mybir.AluOpType.add)
                        tile_i += te - tb

                # post-scale, W2, +b2, residual, LN, relu -> hT (in place)
                for (off, n) in mm_chunks:
                    aggT = chunkp.tile([128, 512], F32, tag="aggT")
                    for kk in range(0, n, 128):
                        k = (off + kk) // 128
                        sc = smallp.tile([128, 128], F32, tag="aggsc")
                        nc.vector.tensor_scalar(sc[:],
                                                agg[:, k * 128:(k + 1) * 128],
                                                inv_sb[:, k:k + 1], None,
                                                op0=mybir.AluOpType.mult)
                        pt = ptr.tile([128, 128], F32, tag="pstr")
                        nc.tensor.transpose(pt[:], sc[:], ident_sb[:])
                        nc.scalar.activation(aggT[:, kk:kk + 128], pt[:],
                                             mybir.ActivationFunctionType.Copy)
                    ps = pmm.tile([128, 512], F32, tag="psmm")
                    nc.tensor.matmul(ps[:, :n], W2l, aggT[:, :n],
                                     start=True, stop=True)
                    zT = chunkp.tile([128, 512], F32, tag="zT")
                    nc.vector.tensor_scalar(zT[:, :n], ps[:, :n],
                                            b2_sb[:, li:li + 1], None,
                                            op0=mybir.AluOpType.add)
                    nc.vector.tensor_tensor(zT[:, :n], zT[:, :n],
                                            hT[:, off:off + n],
                                            op=mybir.AluOpType.add)
                    for kk in range(0, n, 128):
                        k = (off + kk) // 128
                        pz = ptr.tile([128, 128], F32, tag="pstr")
                        nc.tensor.transpose(pz[:], zT[:, kk:kk + 128],
                                            ident_sb[:])
                        layer_norm_tile(
                            pz, k,
                            lng_sb[:, li * 128:(li + 1) * 128],
                            lnb_sb[:, li * 128:(li + 1) * 128],
                            relu=True,
                            dest_ap=hT[:, k * 128:(k + 1) * 128])

            # ---- final LN (in place) + output projection ----
            for k in range(c.NW):
                pz = ptr.tile([128, 128], F32, tag="pstr")
                nc.tensor.transpose(pz[:], hT[:, k * 128:(k + 1) * 128],
                                    ident_sb[:])
                layer_norm_tile(pz, k, outg_sb[:], outb_sb[:], relu=False,
                                dest_ap=hT[:, k * 128:(k + 1) * 128])
            for (off, n) in mm_chunks:
                ps = pmm.tile([128, 512], F32, tag="psmm")
                nc.tensor.matmul(ps[:c.OUT, :n], Wout_sb[:], hT[:, off:off + n],
                                 start=True, stop=True)
                yT = chunkp.tile([c.OUT, 512], F32, tag="yT")
                nc.vector.tensor_scalar(yT[:, :n], ps[:c.OUT, :n],
                                        bout_sb[:, :1],
                                        None, op0=mybir.AluOpType.add)
                for kk in range(0, n, 128):
                    k = (off + kk) // 128
                    py = ptr.tile([128, 128], F32, tag="pstr")
                    nc.tensor.transpose(py[:, :c.OUT], yT[:, kk:kk + 128],
                                        ident_sb[:c.OUT, :c.OUT])
                    ysb = smallp.tile([128, c.OUT], F32, tag="ysb")
                    nc.scalar.activation(ysb[:], py[:, :c.OUT],
                                         mybir.ActivationFunctionType.Copy)
                    nc.sync.dma_start(y_d[k * 128:(k + 1) * 128, :], ysb[:])

    nc.compile()
    _split_multi_waits(nc)
    return nc


# ---------------------------------------------------------------------------
def make_in_maps(cfg: Cfg, plan: EdgePlan, x, degree, Win, b_in, W1, b1, W2,
                 b2, ln_g, ln_b, out_g, out_b, Wout, b_out):
    c = cfg
    inv = (1.0 / np.sqrt(np.asarray(degree, np.float32) + 1.0)).astype(np.float32)
    iota = np.tile(np.arange(128, dtype=np.float32)[None, :], (128, 1)).astype(c.table_np)
    ident = np.eye(128, dtype=np.float32)
    W1s = np.asarray(W1, np.float32).reshape(c.L * 128, c.HID)
    W2s = np.asarray(W2, np.float32).reshape(c.L * 128, c.EMB)
    b1s = np.zeros((128, c.L), np.float32)
    b1s[:c.HID, :] = np.asarray(b1, np.float32).T
    b2s = np.zeros((128, c.L), np.float32)
    b2s[:c.EMB, :] = np.asarray(b2, np.float32).T
    lng = np.concatenate([np.tile(np.asarray(ln_g[i], np.float32)[None, :],
                                  (128, 1)) for i in range(c.L)], axis=1)
    lnb = np.concatenate([np.tile(np.asarray(ln_b[i], np.float32)[None, :],
                                  (128, 1)) for i in range(c.L)], axis=1)
    outg = np.tile(np.asarray(out_g, np.float32)[None, :], (128, 1))
    outb = np.tile(np.asarray(out_b, np.float32)[None, :], (128, 1))
    x = np.asarray(x, np.float32)

    common = dict(
        iota=np.ascontiguousarray(iota),
        ident=ident,
        Win=np.asarray(Win, np.float32),
        b_in=np.asarray(b_in, np.float32)[:, None],
        W1=W1s, b1=b1s, W2=W2s, b2=b2s,
        lng=np.ascontiguousarray(lng), lnb=np.ascontiguousarray(lnb),
        outg=np.ascontiguousarray(outg), outb=np.ascontiguousarray(outb),
        Wout=np.asarray(Wout, np.float32),
        bout=np.asarray(b_out, np.float32)[:, None],
    )
    in_maps = []
    for cc in range(c.NCORES):
        xc = x[cc * c.B_REAL:(cc + 1) * c.B_REAL]
        xpad = np.zeros((c.B, 128), np.float32)
        xpad[:c.B_REAL] = xc
        invc = np.zeros(c.B, np.float32)
        invc[:c.B_REAL] = inv[cc * c.B_REAL:(cc + 1) * c.B_REAL]
        m = dict(common)
        m["xT"] = np.ascontiguousarray(xpad.T)
        m["inv"] = np.ascontiguousarray(invc.reshape(c.NW, 128).T)
        m["gidx"] = np.ascontiguousarray(plan.gidx_w[cc])
        m["ohv"] = np.ascontiguousarray(plan.ohv_sb[cc].astype(c.table_np))
        in_maps.append(m)
    return in_maps


# ---------------------------------------------------------------------------
class Runner:
    """Persistent compiled executable: build once, execute many times."""

    def __init__(self, cfg: Cfg, nc):
        import jax
        import jax.numpy as jnp
        from jax.experimental.shard_map import shard_map
        from jax.sharding import Mesh, PartitionSpec
        import jax.core
        from concourse import bass2jax

        bass2jax.install_neuronx_cc_hook()
        self.cfg = cfg
        self.nc = nc
        partition_name = (nc.partition_id_tensor.name
                          if nc.partition_id_tensor else None)
        in_names, out_names, out_avals, zero_outs = [], [], [], []
        for alloc in nc.m.functions[0].allocations:
            if not isinstance(alloc, mybir.MemoryLocationSet):
                continue
            name = alloc.memorylocations[0].name
            if alloc.kind == "ExternalInput":
                if name != partition_name:
                    in_names.append(name)
            elif alloc.kind == "ExternalOutput":
                out_names.append(name)
                shape = tuple(alloc.tensor_shape)
                dtype = mybir.dt.np(alloc.dtype)
                out_avals.append(jax.core.ShapedArray(shape, dtype))
                zero_outs.append(np.zeros(shape, dtype))
        self.n_params = len(in_names)
        self.in_names = list(in_names)
        self.out_names = out_names
        self.out_avals = out_avals
        all_in = in_names + out_names
        if partition_name is not None:
            all_in.append(partition_name)

        def _body(*args):
            operands = list(args)
            if partition_name is not None:
                operands.append(bass2jax.partition_id_tensor())
            outs = bass2jax._bass_exec_p.bind(
                *operands,
                out_avals=tuple(out_avals),
                in_names=tuple(all_in),
                out_names=tuple(out_names),
                lowering_input_output_aliases=(),
                sim_require_finite=True,
                sim_require_nnan=True,
                nc=nc,
            )
            return tuple(outs)

        devices = jax.devices()[:cfg.NCORES]
        mesh = Mesh(np.asarray(devices), ("core",))
        n_all = self.n_params + len(out_names)
        self.sharded = jax.jit(
            shard_map(_body, mesh=mesh,
                      in_specs=(PartitionSpec("core"),) * n_all,
                      out_specs=(PartitionSpec("core"),) * len(out_names),
                      check_rep=False),
            keep_unused=True,
        )
        self.zero_concat = [
            jax.device_put(
                np.zeros((cfg.NCORES * z.shape[0], *z.shape[1:]), z.dtype))
            for z in zero_outs
        ]
        self._dev_inputs = None

    def set_inputs(self, in_maps):
        import jax
        concat_in = [
            np.concatenate([np.asarray(in_maps[cc][name])
                            for cc in range(self.cfg.NCORES)], axis=0)
            for name in self.in_names[:self.n_params]
        ]
        self._dev_inputs = [jax.device_put(a) for a in concat_in]

    def execute(self):
        import jax
        outs = self.sharded(*self._dev_inputs, *self.zero_concat)
        jax.block_until_ready(outs)
        return outs

    def output(self, outs):
        c = self.cfg
        i = self.out_names.index("y")
        arr = np.asarray(outs[i]).reshape(c.NCORES, *self.out_avals[i].shape)
        return np.concatenate([arr[cc][:c.B_REAL] for cc in range(c.NCORES)],
                              axis=0)


_CACHE = {}


def get_runner(cfg: Cfg, edge_index):
    key = ("plan", cfg.N, cfg.E, str(cfg.table_dt), cfg.CH)
    ek = np.asarray(edge_index)
    if key not in _CACHE or not np.array_equal(_CACHE[key][0], ek):
        plan = EdgePlan(cfg, ek)
        nc = build_program(cfg, plan)
        runner = Runner(cfg, nc)
        _CACHE[key] = (ek.copy(), plan, runner)
    return _CACHE[key][1], _CACHE[key][2]


def run(cfg: Cfg, x, edge_index, degree, Win, b_in, W1, b1, W2, b2,
        ln_g, ln_b, out_g, out_b, Wout, b_out):
    plan, runner = get_runner(cfg, edge_index)
    in_maps = make_in_maps(cfg, plan, x, degree, Win, b_in, W1, b1, W2, b2,
                           ln_g, ln_b, out_g, out_b, Wout, b_out)
    runner.set_inputs(in_maps)
    outs = runner.execute()
    return runner.output(outs)


def kernel(x, edge_index, degree, Win, b_in, W1, b1, W2, b2,
           ln_g, ln_b, out_g, out_b, Wout, b_out):
    cfg = Cfg()
    return run(cfg, x, edge_index, degree, Win, b_in, W1, b1, W2, b2,
               ln_g, ln_b, out_g, out_b, Wout, b_out)


# revision 22
# speedup vs baseline: 1.2367x; 1.2367x over previous
"""BoundaryGCN on 8 Trainium2 NeuronCores.

Strategy (dst-sharded, memory-roofline oriented):
  - Nodes are row-sharded by id across the 8 cores (12500/core, padded to
    12544 = 98*128). All per-node compute (input linear, W1/W2 matmuls,
    LayerNorm, residual, output linear) is data-parallel over node blocks.
  - Per layer, each core computes its block of m = h @ W1 + b1, pre-scales
    rows by inv = rsqrt(deg+1) and AllGathers the scaled table
    (symmetric GCN norm factorizes: coef = inv[src]*inv[dst], so the
    per-edge coefficient disappears entirely).
  - Edges are sharded by destination core. Each core's edges are sorted by
    (src-group, dst-window) host-side; messages are fetched with the SWDGE
    dma_gather custom instruction (512B rows) and segment-summed on the PE
    via one-hot matmuls accumulated in PSUM, then added into an
    SBUF-resident aggregate. Self-loops are folded into the aggregate's
    initialization (exact, no gather traffic).
  - agg is post-scaled by inv, pushed through W2, residual+LN+ReLU, and the
    final LN + output projection produce each core's [12500, 64] slice.

The per-edge index/one-hot metadata is precomputed on the host from
edge_index (it is the same for all 3 layers) and shipped as kernel inputs;
the instruction stream itself is identical across cores (SPMD).
"""
import sys

sys.path.insert(0, "/opt/trn_rl_repo")

import numpy as np

import concourse.bacc as bacc
import concourse.mybir as mybir
import concourse.tile as tile
from concourse.bass_utils import run_bass_kernel_spmd

F32 = mybir.dt.float32
I16 = mybir.dt.int16


# ---------------------------------------------------------------------------
# walrus in this toolchain rejects >1 sync wait per instruction; split extras
# onto same-engine NoOps placed right before the instruction.
def _split_multi_waits(nc, maxw=1):
    n = 0
    for fn in nc.m.functions:
        for bb in fn.blocks:
            insts = bb.instructions
            i = 0
            while i < len(insts):
                ins = insts[i]
                si = ins.sync_info
                if si is None or len(si.on_wait) <= maxw:
                    i += 1
                    continue
                waits = list(si.on_wait)
                keep = waits[-maxw:]
                rest = waits[:-maxw]
                del si.on_wait[:]
                si.on_wait.extend(keep)
                for j in range(0, len(rest), maxw):
                    n += 1
                    nop = mybir.InstNoOp(
                        name=f"wsplit-{n}",
                        sync_info=mybir.SyncInfo(
                            on_wait=list(rest[j:j + maxw]), on_update=[]
                        ),
                        bass_nofuse=True,
                        engine=ins.engine,
                        ins=[],
                        outs=[],
                    )
                    try:
                        nc.register_instruction(nop, overwrite=True)
                    except Exception:
                        pass
                    insts.insert(i, nop)
                    i += 1
                i += 1
    return n


# ---------------------------------------------------------------------------
class Cfg:
    def __init__(self, N=100000, E=1600000, IN_DIM=128, EMB=128, HID=128,
                 OUT=64, L=3, NCORES=8, CH=1024, table_bf16=False):
        assert EMB == 128 and HID == 128 and IN_DIM == 128
        self.N, self.E = N, E
        self.IN_DIM, self.EMB, self.HID, self.OUT, self.L = IN_DIM, EMB, HID, OUT, L
        self.NCORES = NCORES
        self.B_REAL = N // NCORES                      # real nodes per core
        self.NW = (self.B_REAL + 127) // 128           # dst windows per core
        self.B = self.NW * 128                         # padded nodes per core
        self.NPAD = NCORES * self.B
        self.RANKS_PER_GROUP = 2
        self.GROUPS = NCORES // self.RANKS_PER_GROUP
        self.GROUP_ROWS = self.RANKS_PER_GROUP * self.B
        assert self.GROUP_ROWS <= 32767
        self.CH = CH                                   # gather chunk (tokens)
        self.LN_EPS = 1e-5
        self.table_dt = mybir.dt.bfloat16 if table_bf16 else F32
        self.table_np = np.dtype("bfloat16") if table_bf16 else np.float32


class EdgePlan:
    """Host-side uniform (SPMD) token-stream layout for the edge gather."""

    def __init__(self, cfg: Cfg, edge_index: np.ndarray):
        c = cfg
        src = np.asarray(edge_index[0], dtype=np.int64)
        dst = np.asarray(edge_index[1], dtype=np.int64)
        core = dst // c.B_REAL
        ldst = dst - core * c.B_REAL
        w = ldst >> 7                                  # dst window
        ohv = (ldst & 127).astype(np.float32)
        r = src // c.B_REAL
        l = src - r * c.B_REAL
        trow = r * c.B + l
        g = r // c.RANKS_PER_GROUP
        grow = (trow - g * c.GROUP_ROWS).astype(np.int64)

        cnt = np.zeros((c.NCORES, c.GROUPS, c.NW), np.int64)
        np.add.at(cnt, (core, g, w), 1)
        self.Tbar = np.ceil(cnt.max(axis=0) / 128).astype(np.int64)  # [G, NW]

        # global tile index layout: g-major, then w
        tiles_per_gw = self.Tbar                        # [G, NW]
        self.NTILES = int(tiles_per_gw.sum())
        self.TOK = self.NTILES * 128
        tile_start = np.zeros((c.GROUPS, c.NW), np.int64)
        acc = 0
        self.tiles_meta = []  # (g, w, start, stop) per global tile
        for gg in range(c.GROUPS):
            for ww in range(c.NW):
                tile_start[gg, ww] = acc
                T = int(tiles_per_gw[gg, ww])
                for t in range(T):
                    self.tiles_meta.append((gg, ww, t == 0, t == T - 1))
                acc += T
        self.group_tok = tiles_per_gw.sum(axis=1) * 128  # tokens per group
        self.tok_start = tile_start * 128               # [G, NW] token offset

        # per-core token arrays
        order = np.lexsort((src, w, g, core))
        core_s, g_s, w_s, grow_s, ohv_s = (
            core[order], g[order], w[order], grow[order], ohv[order])
        self.gidx = np.zeros((c.NCORES, self.TOK), np.int16)
        self.ohv = np.full((c.NCORES, self.TOK), -1.0, np.float32)
        # segment offsets per (core, g, w)
        seg_cnt = cnt  # [NCORES, G, NW]
        for cc in range(c.NCORES):
            sel = core_s == cc
            gg_, ww_, gr_, ov_ = g_s[sel], w_s[sel], grow_s[sel], ohv_s[sel]
            # within the sorted order, edges of (g,w) are contiguous
            seg_sizes = seg_cnt[cc].ravel()             # g-major, w within
            starts = self.tok_start.ravel()
            pos = np.repeat(starts, seg_sizes) + _ranges(seg_sizes)
            self.gidx[cc, pos] = gr_.astype(np.int16)
            self.ohv[cc, pos] = ov_

        # chunk split per group
        self.chunks = []  # (idx_free_off, tok_off, n_tok, group)
        idx_off = 0
        tok_off = 0
        for gg in range(c.GROUPS):
            rem = int(self.group_tok[gg])
            while rem > 0:
                n = min(c.CH, rem)
                self.chunks.append((idx_off, tok_off, n, gg))
                idx_off += n // 16
                tok_off += n
                rem -= n

        # wrapped idx layout [128, TOK//16]: per chunk block [16, n/16]
        # replicated 8x over partitions
        self.gidx_w = np.zeros((c.NCORES, 128, self.TOK // 16), np.int16)
        for (ioff, toff, n, gg) in self.chunks:
            blk = self.gidx[:, toff:toff + n].reshape(c.NCORES, n // 16, 16)
            blk = np.transpose(blk, (0, 2, 1))          # [NCORES, 16, n/16]
            self.gidx_w[:, :, ioff:ioff + n // 16] = np.tile(blk, (1, 8, 1))
        # ohv arranged [128, NTILES]
        self.ohv_sb = np.transpose(
            self.ohv.reshape(c.NCORES, self.NTILES, 128), (0, 2, 1)).copy()


def _ranges(sizes):
    """concat(arange(s) for s in sizes)"""
    total = int(sizes.sum())
    out = np.arange(total, dtype=np.int64)
    ends = np.cumsum(sizes)
    starts = ends - sizes
    out -= np.repeat(starts, sizes)
    return out


# ---------------------------------------------------------------------------
def build_program(cfg: Cfg, plan: EdgePlan, sim_nocc=False):
    c = cfg
    TD = c.table_dt
    nc = bacc.Bacc("TRN2", target_bir_lowering=False, debug=False,
                   num_devices=c.NCORES, num_swdge_queues=4)

    # ---- kernel I/O ----
    xT_d = nc.declare_dram_parameter("xT", [128, c.B], F32, isOutput=False)
    inv_d = nc.declare_dram_parameter("inv", [128, c.NW], F32, isOutput=False)
    gidx_d = nc.declare_dram_parameter("gidx", [128, plan.TOK // 16], I16, isOutput=False)
    ohv_d = nc.declare_dram_parameter("ohv", [128, plan.NTILES], TD, isOutput=False)
    iota_d = nc.declare_dram_parameter("iota", [128, 128], TD, isOutput=False)
    ident_d = nc.declare_dram_parameter("ident", [128, 128], F32, isOutput=False)
    Win_d = nc.declare_dram_parameter("Win", [128, c.EMB], F32, isOutput=False)
    b_in_d = nc.declare_dram_parameter("b_in", [128, 1], F32, isOutput=False)
    W1_d = nc.declare_dram_parameter("W1", [c.L * 128, c.HID], F32, isOutput=False)
    b1_d = nc.declare_dram_parameter("b1", [128, c.L], F32, isOutput=False)
    W2_d = nc.declare_dram_parameter("W2", [c.L * 128, c.EMB], F32, isOutput=False)
    b2_d = nc.declare_dram_parameter("b2", [128, c.L], F32, isOutput=False)
    lng_d = nc.declare_dram_parameter("lng", [128, c.L * 128], F32, isOutput=False)
    lnb_d = nc.declare_dram_parameter("lnb", [128, c.L * 128], F32, isOutput=False)
    outg_d = nc.declare_dram_parameter("outg", [128, 128], F32, isOutput=False)
    outb_d = nc.declare_dram_parameter("outb", [128, 128], F32, isOutput=False)
    Wout_d = nc.declare_dram_parameter("Wout", [128, c.OUT], F32, isOutput=False)
    bout_d = nc.declare_dram_parameter("bout", [c.OUT, 1], F32, isOutput=False)
    y_d = nc.declare_dram_parameter("y", [c.B, c.OUT], F32, isOutput=True)

    # chunking of the node dim for 512-wide matmuls
    mm_chunks = []
    off = 0
    while off < c.B:
        n = min(512, c.B - off)
        mm_chunks.append((off, n))
        off += n

    with tile.TileContext(nc) as tc:
        with (
            tc.tile_pool(name="const", bufs=1) as constp,
            tc.tile_pool(name="big", bufs=1) as bigp,
            tc.tile_pool(name="chunkp", bufs=3) as chunkp,
            tc.tile_pool(name="datap", bufs=2) as datap,
            tc.tile_pool(name="idxp", bufs=2) as idxp,
            tc.tile_pool(name="ohp", bufs=2) as ohp,
            tc.tile_pool(name="smallp", bufs=3) as smallp,
            tc.tile_pool(name="colp", bufs=3) as colp,
            tc.tile_pool(name="pmm", bufs=2, space="PSUM") as pmm,
            tc.tile_pool(name="pacc", bufs=2, space="PSUM") as pacc,
            tc.tile_pool(name="ptr", bufs=3, space="PSUM") as ptr,
            tc.tile_pool(name="dram", bufs=1, space="DRAM") as dramp,
        ):
            # ---- persistent SBUF ----
            hT = bigp.tile([128, c.B], F32, tag="hT")
            agg = bigp.tile([128, c.B], F32, tag="agg")
            ohv_sb = bigp.tile([128, plan.NTILES], TD, tag="ohv")
            inv_sb = constp.tile([128, c.NW], F32)
            iota_sb = constp.tile([128, 128], TD)
            ident_sb = constp.tile([128, 128], F32)
            Win_sb = constp.tile([128, c.EMB], F32)
            b_in_sb = constp.tile([128, 1], F32)
            W1_sb = constp.tile([128, c.L * c.HID], F32)
            b1_sb = constp.tile([128, c.L], F32)
            W2_sb = constp.tile([128, c.L * c.EMB], F32)
            b2_sb = constp.tile([128, c.L], F32)
            lng_sb = constp.tile([128, c.L * 128], F32)
            lnb_sb = constp.tile([128, c.L * 128], F32)
            outg_sb = constp.tile([128, 128], F32)
            outb_sb = constp.tile([128, 128], F32)
            Wout_sb = constp.tile([128, c.OUT], F32)
            bout_sb = constp.tile([c.OUT, 1], F32)
            eps_sb = constp.tile([128, 1], F32)
            nc.vector.memset(eps_sb[:], c.LN_EPS)

            nc.sync.dma_start(inv_sb[:], inv_d[:])
            nc.sync.dma_start(iota_sb[:], iota_d[:])
            nc.sync.dma_start(ident_sb[:], ident_d[:])
            nc.sync.dma_start(Win_sb[:], Win_d[:])
            nc.sync.dma_start(b_in_sb[:], b_in_d[:])
            for i in range(c.L):
                nc.sync.dma_start(W1_sb[:, i * c.HID:(i + 1) * c.HID],
                                  W1_d[i * 128:(i + 1) * 128, :])
                nc.sync.dma_start(W2_sb[:, i * c.EMB:(i + 1) * c.EMB],
                                  W2_d[i * 128:(i + 1) * 128, :])
            nc.sync.dma_start(b1_sb[:], b1_d[:])
            nc.sync.dma_start(b2_sb[:], b2_d[:])
            nc.sync.dma_start(lng_sb[:], lng_d[:])
            nc.sync.dma_start(lnb_sb[:], lnb_d[:])
            nc.sync.dma_start(outg_sb[:], outg_d[:])
            nc.sync.dma_start(outb_sb[:], outb_d[:])
            nc.sync.dma_start(Wout_sb[:], Wout_d[:])
            nc.sync.dma_start(bout_sb[:], bout_d[:])
            nc.sync.dma_start(ohv_sb[:], ohv_d[:])

            # ---- DRAM tiles for the collective (Shared tiles are
            # single-write: one per layer) ----
            cc_ins = [dramp.tile([c.B, 128], TD, tag=f"cc_in{i}",
                                 name=f"cc_in{i}") for i in range(c.L)]
            m_tables = [dramp.tile([c.NPAD, 128], TD, addr_space="Shared",
                                   tag=f"m_table{i}", name=f"m_table{i}")
                        for i in range(c.L)]

            # ---- layer 0: hT = relu(x @ Win + b_in), feature-major ----
            for (off, n) in mm_chunks:
                xt = chunkp.tile([128, 512], F32, tag="xt")
                nc.sync.dma_start(xt[:, :n], xT_d[:, off:off + n])
                ps = pmm.tile([128, 512], F32, tag="psmm")
                nc.tensor.matmul(ps[:, :n], Win_sb[:], xt[:, :n],
                                 start=True, stop=True)
                nc.scalar.activation(hT[:, off:off + n], ps[:, :n],
                                     mybir.ActivationFunctionType.Relu,
                                     bias=b_in_sb[:, :1], scale=1.0)

            def layer_norm_tile(z_ps, k, g_ap, b_ap, relu, dest_ap):
                """z_ps: [128,128] PSUM node-major. Writes dest_ap (feature
                major [128,128] slice of an SBUF tensor) = T(LN(z)) (+relu)."""
                sum_ = colp.tile([128, 1], F32, tag="lnsum")
                nc.vector.tensor_reduce(sum_[:], z_ps[:], mybir.AxisListType.X,
                                        mybir.AluOpType.add)
                mean = colp.tile([128, 1], F32, tag="lnmean")
                nc.vector.tensor_scalar_mul(mean[:], sum_[:], 1.0 / 128.0)
                cen = smallp.tile([128, 128], F32, tag="lncen")
                nc.vector.tensor_scalar(cen[:], z_ps[:], mean[:, :1], None,
                                        op0=mybir.AluOpType.subtract)
                sq = smallp.tile([128, 128], F32, tag="lnsq")
                ss = colp.tile([128, 1], F32, tag="lnss")
                nc.scalar.activation(sq[:], cen[:],
                                     mybir.ActivationFunctionType.Square,
                                     accum_out=ss[:])
                sd = colp.tile([128, 1], F32, tag="lnsd")
                nc.scalar.activation(sd[:], ss[:],
                                     mybir.ActivationFunctionType.Sqrt,
                                     bias=eps_sb[:, :1], scale=1.0 / 128.0)
                rstd = colp.tile([128, 1], F32, tag="lnrstd")
                nc.vector.reciprocal(rstd[:], sd[:])
                nrm = smallp.tile([128, 128], F32, tag="lnnrm")
                nc.vector.tensor_scalar(nrm[:], cen[:], rstd[:, :1], None,
                                        op0=mybir.AluOpType.mult)
                ng = smallp.tile([128, 128], F32, tag="lnng")
                nc.vector.tensor_tensor(ng[:], nrm[:], g_ap,
                                        op=mybir.AluOpType.mult)
                hnew = smallp.tile([128, 128], F32, tag="lnhnew")
                nc.vector.tensor_tensor(hnew[:], ng[:], b_ap,
                                        op=mybir.AluOpType.add)
                pt = ptr.tile([128, 128], F32, tag="pstr")
                nc.tensor.transpose(pt[:], hnew[:], ident_sb[:])
                nc.scalar.activation(
                    dest_ap, pt[:],
                    mybir.ActivationFunctionType.Relu if relu
                    else mybir.ActivationFunctionType.Copy)

            # ---- GCN layers ----
            for li in range(c.L):
                W1l = W1_sb[:, li * c.HID:(li + 1) * c.HID]
                W2l = W2_sb[:, li * c.EMB:(li + 1) * c.EMB]
                cc_in = cc_ins[li]
                m_table = m_tables[li]

                # m^T chunks, bias, transpose to node-major, scale by inv,
                # write agg init and the AllGather input
                for (off, n) in mm_chunks:
                    ps = pmm.tile([128, 512], F32, tag="psmm")
                    nc.tensor.matmul(ps[:, :n], W1l, hT[:, off:off + n],
                                     start=True, stop=True)
                    mT = chunkp.tile([128, 512], F32, tag="mT")
                    nc.vector.tensor_scalar(mT[:, :n], ps[:, :n],
                                            b1_sb[:, li:li + 1], None,
                                            op0=mybir.AluOpType.add)
                    for kk in range(0, n, 128):
                        k = (off + kk) // 128
                        pt = ptr.tile([128, 128], F32, tag="pstr")
                        nc.tensor.transpose(pt[:], mT[:, kk:kk + 128],
                                            ident_sb[:])
                        # agg init = inv * m   (self-loop term, f32)
                        nc.vector.tensor_scalar(
                            agg[:, k * 128:(k + 1) * 128], pt[:],
                            inv_sb[:, k:k + 1], None,
                            op0=mybir.AluOpType.mult)
                        if TD == F32:
                            nc.sync.dma_start(
                                cc_in[k * 128:(k + 1) * 128, :],
                                agg[:, k * 128:(k + 1) * 128])
                        else:
                            mb = smallp.tile([128, 128], TD, tag="mbf")
                            nc.vector.tensor_scalar(
                                mb[:], pt[:], inv_sb[:, k:k + 1], None,
                                op0=mybir.AluOpType.mult)
                            nc.sync.dma_start(
                                cc_in[k * 128:(k + 1) * 128, :], mb[:])

                if sim_nocc:
                    nc.sync.dma_start(m_table[:c.B, :], cc_in[:])
                else:
                    nc.gpsimd.collective_compute(
                        "AllGather", mybir.AluOpType.bypass,
                        ins=[cc_in.opt()], outs=[m_table.opt()],
                        replica_groups=[list(range(c.NCORES))])

                # gather + one-hot segment-sum
                tile_i = 0
                cur_ps = None
                for ci, (ioff, toff, n_tok, gg) in enumerate(plan.chunks):
                    it = idxp.tile([128, c.CH // 16], I16, tag="idx")
                    nc.sync.dma_start(it[:, :n_tok // 16],
                                      gidx_d[:, ioff:ioff + n_tok // 16])
                    dt_ = datap.tile([128, c.CH // 128, 128], TD, tag="gdat")
                    nc.gpsimd.dma_gather(
                        dt_[:, :n_tok // 128, :],
                        m_table[gg * c.GROUP_ROWS:(gg + 1) * c.GROUP_ROWS, :],
                        it[:, :n_tok // 16],
                        num_idxs=n_tok, num_idxs_reg=n_tok, elem_size=128,
                        queue_num=ci % 4)
                    n_t = n_tok // 128
                    OHW = 8
                    for tb in range(0, n_t, OHW):
                        te = min(tb + OHW, n_t)
                        oh = ohp.tile([128, OHW, 128], TD, tag="oh")
                        nc.vector.tensor_tensor(
                            oh[:, :te - tb, :],
                            iota_sb[:].unsqueeze(1).to_broadcast(
                                [128, te - tb, 128]),
                            ohv_sb[:, tile_i:tile_i + (te - tb)].unsqueeze(2)
                            .to_broadcast([128, te - tb, 128]),
                            op=mybir.AluOpType.is_equal)
                        for t in range(tb, te):
                            (g_, w_, is_start, is_stop) = \
                                plan.tiles_meta[tile_i + t - tb]
                            if is_start:
                                cur_ps = pacc.tile([128, 128], F32,
                                                   tag="psacc")
                            nc.tensor.matmul(cur_ps[:], oh[:, t - tb, :],
                                             dt_[:, t, :],
                                             start=is_start, stop=is_stop)
                            if is_stop:
                                nc.vector.tensor_tensor(
                                    agg[:, w_ * 128:(w_ + 1) * 128],
                                    agg[:, w_ * 128:(w_ + 1) * 128],
                                    cur_ps[:], op=mybir.AluOpType.add)
                        tile_i += te - tb

                # post-scale, W2, +b2, residual, LN, relu -> hT (in place)
                for (off, n) in mm_chunks:
                    aggT = chunkp.tile([128, 512], F32, tag="aggT")
                    for kk in range(0, n, 128):
                        k = (off + kk) // 128
                        sc = smallp.tile([128, 128], F32, tag="aggsc")
                        nc.vector.tensor_scalar(sc[:],
                                                agg[:, k * 128:(k + 1) * 128],
                                                inv_sb[:, k:k + 1], None,
                                                op0=mybir.AluOpType.mult)
                        pt = ptr.tile([128, 128], F32, tag="pstr")
                        nc.tensor.transpose(pt[:], sc[:], ident_sb[:])
                        nc.scalar.activation(aggT[:, kk:kk + 128], pt[:],
                                             mybir.ActivationFunctionType.Copy)
                    ps = pmm.tile([128, 512], F32, tag="psmm")
                    nc.tensor.matmul(ps[:, :n], W2l, aggT[:, :n],
                                     start=True, stop=True)
                    zT = chunkp.tile([128, 512], F32, tag="zT")
                    nc.vector.tensor_scalar(zT[:, :n], ps[:, :n],
                                            b2_sb[:, li:li + 1], None,
                                            op0=mybir.AluOpType.add)
                    nc.vector.tensor_tensor(zT[:, :n], zT[:, :n],
                                            hT[:, off:off + n],
                                            op=mybir.AluOpType.add)
                    for kk in range(0, n, 128):
                        k = (off + kk) // 128
                        pz = ptr.tile([128, 128], F32, tag="pstr")
                        nc.tensor.transpose(pz[:], zT[:, kk:kk + 128],
                                            ident_sb[:])
                        layer_norm_tile(
                            pz, k,
                            lng_sb[:, li * 128:(li + 1) * 128],
                            lnb_sb[:, li * 128:(li + 1) * 128],
                            relu=True,
                            dest_ap=hT[:, k * 128:(k + 1) * 128])

            # ---- final LN (in place) + output projection ----
            for k in range(c.NW):
                pz = ptr.tile([128, 128], F32, tag="pstr")
                nc.tensor.transpose(pz[:], hT[:, k * 128:(k + 1) * 128],
                                    ident_sb[:])
                layer_norm_tile(pz, k, outg_sb[:], outb_sb[:], relu=False,
                                dest_ap=hT[:, k * 128:(k + 1) * 128])
            for (off, n) in mm_chunks:
                ps = pmm.tile([128, 512], F32, tag="psmm")
                nc.tensor.matmul(ps[:c.OUT, :n], Wout_sb[:], hT[:, off:off + n],
                                 start=True, stop=True)
                yT = chunkp.tile([c.OUT, 512], F32, tag="yT")
                nc.vector.tensor_scalar(yT[:, :n], ps[:c.OUT, :n],
                                        bout_sb[:, :1],
                                        None, op0=mybir.AluOpType.add)
                for kk in range(0, n, 128):
                    k = (off + kk) // 128
                    py = ptr.tile([128, 128], F32, tag="pstr")
                    nc.tensor.transpose(py[:, :c.OUT], yT[:, kk:kk + 128],
                                        ident_sb[:c.OUT, :c.OUT])
                    ysb = smallp.tile([128, c.OUT], F32, tag="ysb")
                    nc.scalar.activation(ysb[:], py[:, :c.OUT],
                                         mybir.ActivationFunctionType.Copy)
                    nc.sync.dma_start(y_d[k * 128:(k + 1) * 128, :], ysb[:])

    nc.compile()
    _split_multi_waits(nc)
    return nc


# ---------------------------------------------------------------------------
def make_in_maps(cfg: Cfg, plan: EdgePlan, x, degree, Win, b_in, W1, b1, W2,
                 b2, ln_g, ln_b, out_g, out_b, Wout, b_out):
    c = cfg
    inv = (1.0 / np.sqrt(np.asarray(degree, np.float32) + 1.0)).astype(np.float32)
    iota = np.tile(np.arange(128, dtype=np.float32)[None, :], (128, 1)).astype(c.table_np)
    ident = np.eye(128, dtype=np.float32)
    W1s = np.asarray(W1, np.float32).reshape(c.L * 128, c.HID)
    W2s = np.asarray(W2, np.float32).reshape(c.L * 128, c.EMB)
    b1s = np.zeros((128, c.L), np.float32)
    b1s[:c.HID, :] = np.asarray(b1, np.float32).T
    b2s = np.zeros((128, c.L), np.float32)
    b2s[:c.EMB, :] = np.asarray(b2, np.float32).T
    lng = np.concatenate([np.tile(np.asarray(ln_g[i], np.float32)[None, :],
                                  (128, 1)) for i in range(c.L)], axis=1)
    lnb = np.concatenate([np.tile(np.asarray(ln_b[i], np.float32)[None, :],
                                  (128, 1)) for i in range(c.L)], axis=1)
    outg = np.tile(np.asarray(out_g, np.float32)[None, :], (128, 1))
    outb = np.tile(np.asarray(out_b, np.float32)[None, :], (128, 1))
    x = np.asarray(x, np.float32)

    common = dict(
        iota=np.ascontiguousarray(iota),
        ident=ident,
        Win=np.asarray(Win, np.float32),
        b_in=np.asarray(b_in, np.float32)[:, None],
        W1=W1s, b1=b1s, W2=W2s, b2=b2s,
        lng=np.ascontiguousarray(lng), lnb=np.ascontiguousarray(lnb),
        outg=np.ascontiguousarray(outg), outb=np.ascontiguousarray(outb),
        Wout=np.asarray(Wout, np.float32),
        bout=np.asarray(b_out, np.float32)[:, None],
    )
    in_maps = []
    for cc in range(c.NCORES):
        xc = x[cc * c.B_REAL:(cc + 1) * c.B_REAL]
        xpad = np.zeros((c.B, 128), np.float32)
        xpad[:c.B_REAL] = xc
        invc = np.zeros(c.B, np.float32)
        invc[:c.B_REAL] = inv[cc * c.B_REAL:(cc + 1) * c.B_REAL]
        m = dict(common)
        m["xT"] = np.ascontiguousarray(xpad.T)
        m["inv"] = np.ascontiguousarray(invc.reshape(c.NW, 128).T)
        m["gidx"] = np.ascontiguousarray(plan.gidx_w[cc])
        m["ohv"] = np.ascontiguousarray(plan.ohv_sb[cc].astype(c.table_np))
        in_maps.append(m)
    return in_maps


# ---------------------------------------------------------------------------
class Runner:
    """Persistent compiled executable: build once, execute many times."""

    def __init__(self, cfg: Cfg, nc):
        import jax
        import jax.numpy as jnp
        from jax.experimental.shard_map import shard_map
        from jax.sharding import Mesh, PartitionSpec
        import jax.core
        from concourse import bass2jax

        bass2jax.install_neuronx_cc_hook()
        self.cfg = cfg
        self.nc = nc
        partition_name = (nc.partition_id_tensor.name
                          if nc.partition_id_tensor else None)
        in_names, out_names, out_avals, zero_outs = [], [], [], []
        for alloc in nc.m.functions[0].allocations:
            if not isinstance(alloc, mybir.MemoryLocationSet):
                continue
            name = alloc.memorylocations[0].name
            if alloc.kind == "ExternalInput":
                if name != partition_name:
                    in_names.append(name)
            elif alloc.kind == "ExternalOutput":
                out_names.append(name)
                shape = tuple(alloc.tensor_shape)
                dtype = mybir.dt.np(alloc.dtype)
                out_avals.append(jax.core.ShapedArray(shape, dtype))
                zero_outs.append(np.zeros(shape, dtype))
        self.n_params = len(in_names)
        self.in_names = list(in_names)
        self.out_names = out_names
        self.out_avals = out_avals
        all_in = in_names + out_names
        if partition_name is not None:
            all_in.append(partition_name)

        def _body(*args):
            operands = list(args)
            if partition_name is not None:
                operands.append(bass2jax.partition_id_tensor())
            outs = bass2jax._bass_exec_p.bind(
                *operands,
                out_avals=tuple(out_avals),
                in_names=tuple(all_in),
                out_names=tuple(out_names),
                lowering_input_output_aliases=(),
                sim_require_finite=True,
                sim_require_nnan=True,
                nc=nc,
            )
            return tuple(outs)

        devices = jax.devices()[:cfg.NCORES]
        mesh = Mesh(np.asarray(devices), ("core",))
        n_all = self.n_params + len(out_names)
        self.sharded = jax.jit(
            shard_map(_body, mesh=mesh,
                      in_specs=(PartitionSpec("core"),) * n_all,
                      out_specs=(PartitionSpec("core"),) * len(out_names),
                      check_rep=False),
            keep_unused=True,
        )
        self.zero_concat = [
            jax.device_put(
                np.zeros((cfg.NCORES * z.shape[0], *z.shape[1:]), z.dtype))
            for z in zero_outs
        ]
        self._dev_inputs = None

    def set_inputs(self, in_maps):
        import jax
        concat_in = [
            np.concatenate([np.asarray(in_maps[cc][name])
                            for cc in range(self.cfg.NCORES)], axis=0)
            for name in self.in_names[:self.n_params]
        ]
        self._dev_inputs = [jax.device_put(a) for a in concat_in]

    def execute(self):
        import jax
        outs = self.sharded(*self._dev_inputs, *self.zero_concat)
        jax.block_until_ready(outs)
        return outs

    def output(self, outs):
        c = self.cfg
        i = self.out_names.index("y")
        arr = np.asarray(outs[i]).reshape(c.NCORES, *self.out_avals[i].shape)
        return np.concatenate([arr[cc][:c.B_REAL] for cc in range(c.NCORES)],
                              axis=0)


_CACHE = {}


def get_runner(cfg: Cfg, edge_index):
    key = ("plan", cfg.N, cfg.E, str(cfg.table_dt), cfg.CH)
    ek = np.asarray(edge_index)
    if key not in _CACHE or not np.array_equal(_CACHE[key][0], ek):
        plan = EdgePlan(cfg, ek)
        nc = build_program(cfg, plan)
        runner = Runner(cfg, nc)
        _CACHE[key] = (ek.copy(), plan, runner)
    return _CACHE[key][1], _CACHE[key][2]


def run(cfg: Cfg, x, edge_index, degree, Win, b_in, W1, b1, W2, b2,
        ln_g, ln_b, out_g, out_b, Wout, b_out):
    plan, runner = get_runner(cfg, edge_index)
    in_maps = make_in_maps(cfg, plan, x, degree, Win, b_in, W1, b1, W2, b2,
                           ln_g, ln_b, out_g, out_b, Wout, b_out)
    runner.set_inputs(in_maps)
    outs = runner.execute()
    return runner.output(outs)


def kernel(x, edge_index, degree, Win, b_in, W1, b1, W2, b2,
           ln_g, ln_b, out_g, out_b, Wout, b_out):
    cfg = Cfg()
    return run(cfg, x, edge_index, degree, Win, b_in, W1, b1, W2, b2,
               ln_g, ln_b, out_g, out_b, Wout, b_out)
